# revision 15
# baseline (speedup 1.0000x reference)
"""Trainium2 Bass kernel for nn_Conv2DExperimental (MVN-sampled 3x3 conv).

Computation (per the nn.Module):
  L    = tril(weight_L, -1) + softplus(diag(weight_L)) * I      # [O,I,D,D], D=9
  w    = weight_loc + einsum('oiab,oib->oia', L, eps_w)         # [O,I,3,3]
  b    = bias_loc + eps_b * softplus(bias_ro)                   # [O]
  out  = conv2d(x, w, SAME, NCHW) + b

Distribution: data-parallel over the batch dim of x (32 images -> 8 cores x 4),
with the weight sampling replicated on every core (it is tiny).

Per-core kernel:
  - sampling runs on VectorE/ScalarE with O on the partition dim; the 9 sampled
    64x64 tap matrices are PE-transposed into block-diagonal [128,128] lhsT
    tiles pairing two images per matmul (partitions = (image, channel)).
  - conv runs as 9 shift-matmuls per 2-output-row PSUM tile in float32r
    (fp32 data truncated to FP22 in the PE; 1 cycle/row at N>=256).
  - ScalarE evacuates PSUM with the bias add fused; DMA engines stream
    row-strips of x in and finished strips of out back to HBM.
"""

import sys
from contextlib import ExitStack

for _p in ("/opt/trn_rl_repo",):
    if _p not in sys.path:
        sys.path.insert(0, _p)

import numpy as np

import concourse.bass as bass
import concourse.bacc as bacc
import concourse.mybir as mybir
from concourse.tile import TileContext

F32 = mybir.dt.float32
F32R = mybir.dt.float32r
AF = mybir.ActivationFunctionType

N_CORES = 8
O = 64
I = 64
KK = 3
D = KK * KK  # 9


def build_nc(nb=4, hh=224, ww=224, rstrip=28, x_bufs=3, o_bufs=2):
    """Build the per-core Bass program.

    nb: images per core (must be even: images are processed in pairs)
    hh, ww: spatial dims; rstrip: output rows per strip (must divide hh, even)
    """
    assert nb % 2 == 0 and hh % rstrip == 0 and rstrip % 2 == 0
    wpad = ww + 2
    nstrips = hh // rstrip
    ntiles = rstrip // 2  # psum tiles (2 output rows each) per strip

    nc = bacc.Bacc("TRN2", target_bir_lowering=False, debug=False)

    x_t = nc.dram_tensor("x", [nb, I, hh, ww], F32R, kind="ExternalInput").ap()
    wl_t = nc.dram_tensor("wL", [O, I * D * D], F32, kind="ExternalInput").ap()
    wloc_t = nc.dram_tensor("wloc", [O, I * D], F32, kind="ExternalInput").ap()
    epsw_t = nc.dram_tensor("epsw", [O, I * D], F32, kind="ExternalInput").ap()
    mask_t = nc.dram_tensor("mask", [O, D * D], F32, kind="ExternalInput").ap()
    ident_t = nc.dram_tensor("ident", [O, O], F32, kind="ExternalInput").ap()
    bias3_t = nc.dram_tensor("bias3", [3, O], F32, kind="ExternalInput").ap()
    out_t = nc.dram_tensor("out", [nb, O, hh, ww], F32, kind="ExternalOutput").ap()

    with TileContext(nc) as tc, ExitStack() as stack:
        # ---------------- weight + bias sampling (one-time prologue) --------
        cp = stack.enter_context(tc.tile_pool(name="consts", bufs=1))
        wl = cp.tile([O, I * D * D], F32, name="wl", tag="wl")
        wloc = cp.tile([O, I * D], F32, name="wloc_s", tag="wloc_s")
        epsw = cp.tile([O, I * D], F32, name="epsw_s", tag="epsw_s")
        mask = cp.tile([O, D * D], F32, name="mask_s", tag="mask_s")
        ident = cp.tile([O, O], F32, name="ident_s", tag="ident_s")
        b3 = cp.tile([O, 3], F32, name="b3", tag="b3")
        sp = cp.tile([O, I * D], F32, name="sp", tag="sp")
        tmp = cp.tile([O, I * D], F32, name="tmp", tag="tmp")
        wsamp = cp.tile([O, I * D], F32, name="wsamp", tag="wsamp")
        bias = cp.tile([128, 1], F32, name="bias", tag="bias")
        # 9 block-diagonal lhsT tiles, stored side by side: [128, 9*128]
        wts = cp.tile([128, D * 128], F32R, name="wts", tag="wts")
        sp_b = cp.tile([O, 1], F32, name="sp_b", tag="sp_b")

        nc.sync.dma_start(wl[:], wl_t[:])
        nc.sync.dma_start(wloc[:], wloc_t[:])
        nc.sync.dma_start(epsw[:], epsw_t[:])
        nc.sync.dma_start(mask[:], mask_t[:])
        nc.sync.dma_start(ident[:], ident_t[:])
        nc.sync.dma_start(b3[:], bias3_t.transpose([1, 0]))

        # bias = bias_loc + eps_b * softplus(bias_ro); bias3 rows: loc, ro, eps_b
        # (no Softplus LUT in this toolchain: softplus(x) = ln(exp(x) + 1))
        nc.scalar.activation(sp_b[:], b3[:, 1:2], AF.Exp)
        nc.scalar.activation(sp_b[:], sp_b[:], AF.Ln, bias=1.0)
        nc.vector.tensor_mul(sp_b[:], sp_b[:], b3[:, 2:3])
        nc.vector.tensor_add(bias[0:O, :], b3[:, 0:1], sp_b[:])
        nc.sync.dma_start(bias[O:128, :], bias[0:O, :])

        # softplus of the per-(o,i) diagonals: wl free layout is (i, d=a*9+b);
        # diagonal entries sit at d = 10*a  ->  sp layout (i, a)
        wl3 = wl[:].rearrange("o (i d) -> o i d", i=I)
        diag_view = bass.AP(
            tensor=wl[:].tensor,
            offset=wl[:].offset,
            ap=[list(p) for p in wl[:].ap[:1]] + [[D * D, I], [D + 1, D]],
        )
        sp3 = sp[:].rearrange("o (i a) -> o i a", i=I)
        nc.scalar.activation(sp3, diag_view, AF.Exp)
        nc.scalar.activation(sp[:], sp[:], AF.Ln, bias=1.0)

        # strictly-lower mask applied in place (broadcast mask over i)
        mask_b = bass.AP(
            tensor=mask[:].tensor,
            offset=mask[:].offset,
            ap=[list(p) for p in mask[:].ap[:1]] + [[0, I], [1, D * D]],
        )
        nc.vector.tensor_mul(wl3, wl3, mask_b)

        # wsamp = wloc + softplus(diag) * eps  (the b == a term of L @ eps)
        eps3 = epsw[:].rearrange("o (i b) -> o i b", i=I)
        nc.vector.tensor_mul(tmp[:], sp[:], epsw[:])
        nc.vector.tensor_add(wsamp[:], wloc[:], tmp[:])

        # += strict-lower part: for each b, wsamp[o,(i,a)] += wl[o,(i,a*9+b)] * eps[o,(i,b)]
        tmp3 = tmp[:].rearrange("o (i a) -> o i a", i=I)
        for b in range(D - 1):  # b == 8 is never strictly below the diagonal
            wl_b = bass.AP(
                tensor=wl[:].tensor,
                offset=wl[:].offset + b,
                ap=[list(p) for p in wl[:].ap[:1]] + [[D * D, I], [D, D]],
            )
            eps_b = bass.AP(
                tensor=epsw[:].tensor,
                offset=epsw[:].offset + b,
                ap=[list(p) for p in epsw[:].ap[:1]] + [[D, I], [0, D]],
            )
            nc.vector.tensor_tensor(tmp3, wl_b, eps_b, mybir.AluOpType.mult)
            nc.vector.tensor_add(wsamp[:], wsamp[:], tmp[:])

        # build the 9 block-diagonal lhsT tiles:
        #   wts[:, a*128:(a+1)*128] = [[T_a, 0], [0, T_a]],  T_a[i,o] = wsamp[o, i*9+a]
        nc.vector.memset(wts[:].bitcast(F32), 0.0)
        with tc.tile_pool(name="pt", bufs=2, space="PSUM") as ptp:
            for a in range(D):
                w_a = bass.AP(
                    tensor=wsamp[:].tensor,
                    offset=wsamp[:].offset + a,
                    ap=[list(p) for p in wsamp[:].ap[:1]] + [[D, I]],
                )
                pt = ptp.tile([O, O], F32, name="pt")
                nc.tensor.transpose(pt[:], w_a, ident[:])
                nc.vector.tensor_copy(wts[0:O, a * 128 : a * 128 + O], pt[:])
        # partition-shifted copy of the diagonal blocks: [0:64, a*128:+64] ->
        # [64:128, a*128+64:+64]
        wts_lo = wts[0:O]
        wts_hi = wts[O:128]
        src = bass.AP(
            tensor=wts_lo.tensor,
            offset=wts_lo.offset,
            ap=[list(p) for p in wts_lo.ap[:1]] + [[128, D], [1, O]],
        )
        dst = bass.AP(
            tensor=wts_hi.tensor,
            offset=wts_hi.offset + O,
            ap=[list(p) for p in wts_hi.ap[:1]] + [[128, D], [1, O]],
        )
        nc.sync.dma_start(dst, src)

        # ---------------- convolution ---------------------------------------
        xp = stack.enter_context(tc.tile_pool(name="xstrip", bufs=x_bufs))
        op = stack.enter_context(tc.tile_pool(name="ostrip", bufs=o_bufs))
        pp = stack.enter_context(tc.tile_pool(name="acc", bufs=8, space="PSUM"))
        if True:
            for pair in range(nb // 2):
                n0 = 2 * pair
                for s in range(nstrips):
                    h0 = s * rstrip
                    xs = xp.tile([128, rstrip + 2, wpad], F32R, name="xs")
                    # zero the left/right halo columns
                    halo = bass.AP(
                        tensor=xs[:].tensor,
                        offset=xs[:].offset,
                        ap=[list(p) for p in xs[:].ap[:1]]
                        + [[wpad, rstrip + 2], [ww + 1, 2]],
                    )
                    nc.vector.memset(halo.bitcast(F32), 0.0)
                    # load input rows [h0-1, h0+rstrip], clipped to the image
                    r_lo = max(h0 - 1, 0)
                    r_hi = min(h0 + rstrip + 1, hh)
                    dst_r0 = r_lo - (h0 - 1)
                    if h0 == 0:
                        nc.vector.memset(xs[:, 0:1, :].bitcast(F32), 0.0)
                    if h0 + rstrip == hh:
                        nc.vector.memset(xs[:, rstrip + 1 : rstrip + 2, :].bitcast(F32), 0.0)
                    src = x_t[n0 : n0 + 2, :, r_lo:r_hi, :].rearrange(
                        "n i h w -> (n i) h w"
                    )
                    nc.sync.dma_start(
                        xs[:, dst_r0 : dst_r0 + (r_hi - r_lo), 1 : ww + 1], src
                    )

                    os_ = op.tile([128, rstrip, ww], F32, name="os_")
                    for j in range(ntiles):
                        acc = pp.tile([128, 2, ww], F32, name="acc")
                        for tap in range(D):
                            dy, dx = tap // 3 - 1, tap % 3 - 1
                            rhs = bass.AP(
                                tensor=xs[:].tensor,
                                offset=xs[:].offset
                                + (2 * j + 1 + dy) * wpad
                                + 1
                                + dx,
                                ap=[list(p) for p in xs[:].ap[:1]]
                                + [[wpad, 2], [1, ww]],
                            )
                            nc.tensor.matmul(
                                acc[:],
                                wts[:, tap * 128 : (tap + 1) * 128],
                                rhs,
                                start=(tap == 0),
                                stop=(tap == D - 1),
                            )
                        nc.scalar.activation(
                            os_[:, 2 * j : 2 * j + 2, :],
                            acc[:],
                            AF.Identity,
                            bias=bias[:, 0:1],
                        )
                    dst = out_t[n0 : n0 + 2, :, h0 : h0 + rstrip, :].rearrange(
                        "n i h w -> (n i) h w"
                    )
                    nc.sync.dma_start(dst, os_[:])

    nc.compile()
    return nc


_CACHED_NC = None


def _host_inputs(x_shard, weight_loc, weight_L, bias_loc, bias_ro, eps_w, eps_b):
    mask = np.tril(np.ones((D, D), np.float32), -1).reshape(1, D * D)
    return {
        "x": np.ascontiguousarray(x_shard, np.float32),
        "wL": np.ascontiguousarray(weight_L.reshape(O, I * D * D), np.float32),
        "wloc": np.ascontiguousarray(weight_loc.reshape(O, I * D), np.float32),
        "epsw": np.ascontiguousarray(eps_w.reshape(O, I * D), np.float32),
        "mask": np.ascontiguousarray(np.repeat(mask, O, 0)),
        "ident": np.eye(O, dtype=np.float32),
        "bias3": np.ascontiguousarray(
            np.stack([bias_loc, bias_ro, eps_b]).astype(np.float32)
        ),
    }


def kernel(x, weight_loc, weight_L, bias_loc, bias_ro, eps_w, eps_b):
    global _CACHED_NC
    from concourse.bass_utils import run_bass_kernel_spmd

    x = np.asarray(x, np.float32)
    nb = x.shape[0] // N_CORES
    if _CACHED_NC is None:
        _CACHED_NC = build_nc(nb=nb)
    nc = _CACHED_NC

    in_maps = [
        _host_inputs(
            x[c * nb : (c + 1) * nb],
            np.asarray(weight_loc),
            np.asarray(weight_L),
            np.asarray(bias_loc),
            np.asarray(bias_ro),
            np.asarray(eps_w),
            np.asarray(eps_b),
        )
        for c in range(N_CORES)
    ]
    res = run_bass_kernel_spmd(nc, in_maps, list(range(N_CORES)))
    return np.concatenate([res.results[c]["out"] for c in range(N_CORES)], axis=0)


# revision 43
# speedup vs baseline: 50956.8316x; 50956.8316x over previous
"""Trainium2 Bass kernel for nn_Conv2DExperimental (MVN-sampled 3x3 conv).

Computation (per the nn.Module):
  L    = tril(weight_L, -1) + softplus(diag(weight_L)) * I      # [O,I,D,D], D=9
  w    = weight_loc + einsum('oiab,oib->oia', L, eps_w)         # [O,I,3,3]
  b    = bias_loc + eps_b * softplus(bias_ro)                   # [O]
  out  = conv2d(x, w, SAME, NCHW) + b

Distribution: data-parallel over the batch dim of x (32 images -> 8 cores x 4),
with the weight sampling replicated on every core (it is tiny).

Per-core kernel:
  - sampling runs on VectorE/ScalarE with O on the partition dim; the 9 sampled
    64x64 tap matrices are PE-transposed into block-diagonal [128,128] lhsT
    tiles pairing two images per matmul (partitions = (image, channel)).
  - conv runs as 9 shift-matmuls per 2-output-row PSUM tile in float32r
    (fp32 data truncated to FP22 in the PE; 1 cycle/row at N>=256).
  - ScalarE evacuates PSUM with the bias add fused; DMA engines stream
    row-strips of x in and finished strips of out back to HBM.
"""

import sys
from contextlib import ExitStack

for _p in ("/opt/trn_rl_repo",):
    if _p not in sys.path:
        sys.path.insert(0, _p)

import numpy as np

import concourse.bass as bass
import concourse.bacc as bacc
import concourse.mybir as mybir
from concourse.tile import TileContext

F32 = mybir.dt.float32
F32R = mybir.dt.float32r
AF = mybir.ActivationFunctionType

N_CORES = 8
O = 64
I = 64
KK = 3
D = KK * KK  # 9


def build_nc(nb=4, hh=224, ww=224, rstrip=28, x_bufs=3, o_bufs=2, passes=1):
    """Build the per-core Bass program.

    nb: images per core (must be even: images are processed in pairs)
    hh, ww: spatial dims; rstrip: output rows per strip (must divide hh, even)
    """
    assert nb % 2 == 0 and hh % rstrip == 0 and rstrip % 2 == 0
    wpad = ww + 2
    nstrips = hh // rstrip
    ntiles = rstrip // 2  # psum tiles (2 output rows each) per strip

    nc = bacc.Bacc("TRN2", target_bir_lowering=False, debug=False)

    x_t = nc.dram_tensor("x", [nb, I, hh, ww], F32R, kind="ExternalInput").ap()
    wl_t = nc.dram_tensor("wL", [O, I * D * D], F32, kind="ExternalInput").ap()
    wloc_t = nc.dram_tensor("wloc", [O, I * D], F32, kind="ExternalInput").ap()
    epsw_t = nc.dram_tensor("epsw", [O, I * D], F32, kind="ExternalInput").ap()
    ident_t = nc.dram_tensor("ident", [O, O], F32, kind="ExternalInput").ap()
    bias3_t = nc.dram_tensor("bias3", [3, O], F32, kind="ExternalInput").ap()
    out_t = nc.dram_tensor("out", [nb, O, hh, ww], F32, kind="ExternalOutput").ap()

    with TileContext(nc) as tc, ExitStack() as stack:
        # ---------------- weight + bias sampling (one-time prologue) --------
        cp = stack.enter_context(tc.tile_pool(name="consts", bufs=1))
        wl = cp.tile([O, I * D * D], F32, name="wl", tag="wl")
        wloc = cp.tile([O, I * D], F32, name="wloc_s", tag="wloc_s")
        epsw = cp.tile([O, I * D], F32, name="epsw_s", tag="epsw_s")
        ident = cp.tile([O, O], F32, name="ident_s", tag="ident_s")
        b3 = cp.tile([O, 3], F32, name="b3", tag="b3")
        sp = cp.tile([O, I * D], F32, name="sp", tag="sp")
        tmp = cp.tile([O, I * D], F32, name="tmp", tag="tmp")
        wsamp = cp.tile([O, I * D], F32, name="wsamp", tag="wsamp")
        bias = cp.tile([128, 1], F32, name="bias", tag="bias")
        # 9 block-diagonal lhsT tiles, stored side by side: [128, 9*128]
        wts = cp.tile([128, D * 128], F32R, name="wts", tag="wts")
        sp_b = cp.tile([O, 1], F32, name="sp_b", tag="sp_b")

        b3p = cp.tile([3, O], F32, name="b3p", tag="b3p")
        nc.sync.dma_start(wl[:], wl_t[:])
        nc.scalar.dma_start(b3p[:], bias3_t[:])
        nc.scalar.dma_start(ident[:], ident_t[:])
        nc.scalar.dma_start(wloc[:], wloc_t[:])
        nc.scalar.dma_start(epsw[:], epsw_t[:])

        # PE warm-up feed: zero tiles via GpSimd (idle queue, no input deps)
        # so the warm-up matmuls can start within ~1us of kernel entry.
        identr = cp.tile([O, O], F32R, name="identr", tag="identr")
        junk = cp.tile([O, 256], F32R, name="junk", tag="junk")
        with tc.high_priority():
            nc.gpsimd.memset(identr[:].bitcast(F32), 0.0)
            nc.gpsimd.memset(junk[:].bitcast(F32), 0.0)
        nc.gpsimd.memset(wts[:].bitcast(F32), 0.0)

        # PE warm-up: the HAM clock gate needs ~3.4us of sustained matmul
        # activity to lift the PE from 1.2 to 2.4 GHz, and re-throttles after
        # ~3.4us idle. One long accumulation group (no inter-matmul
        # semaphores) bridges the PE from kernel entry to the transposes.
        with tc.tile_pool(name="wp", bufs=1, space="PSUM") as wp:
            warm = wp.tile([O, 256], F32, name="warm")
            n_warm = 90
            for k in range(n_warm):
                nc.tensor.matmul(
                    warm[:], identr[:], junk[:],
                    start=(k == 0), stop=(k == n_warm - 1),
                )

            # bias3 arrives as [3, 64]; transpose to [64, 3] on the PE (a
            # partition-major DMA of 64x3 elements costs ~17us in descriptors)
            bp_ps = wp.tile([O, 3], F32, name="bp_ps")
            with tc.high_priority():
                nc.tensor.matmul(
                    bp_ps[:], b3p[:], ident[0:3, 0:3], start=True, stop=True
                )
                nc.vector.tensor_copy(b3[:], bp_ps[:])

        # softplus of the per-(o,i) diagonals: wl free layout is (i, d=a*9+b);
        # diagonal entries sit at d = 10*a  ->  sp layout (i, a).
        # ACT order Exp,Exp,Ln,Ln avoids activation-table reload thrash
        # (each ACT_TABLE_LOAD costs ~1.3us). softplus(x) = ln(exp(x) + 1):
        # there is no Softplus LUT in this toolchain.
        diag_view = bass.AP(
            tensor=wl[:].tensor,
            offset=wl[:].offset,
            ap=[list(p) for p in wl[:].ap[:1]] + [[D * D, I], [D + 1, D]],
        )
        sp3 = sp[:].rearrange("o (i a) -> o i a", i=I)
        with tc.high_priority():
            nc.scalar.activation(sp_b[:], b3[:, 1:2], AF.Exp)
            nc.scalar.activation(sp3, diag_view, AF.Exp)
            nc.scalar.activation(sp[:], sp[:], AF.Ln, bias=1.0)
            nc.scalar.activation(sp_b[:], sp_b[:], AF.Ln, bias=1.0)

        # bias = bias_loc + eps_b * softplus(bias_ro)
        nc.vector.tensor_mul(sp_b[:], sp_b[:], b3[:, 2:3])
        nc.vector.tensor_add(bias[0:O, :], b3[:, 0:1], sp_b[:])
        nc.scalar.dma_start(bias[O:128, :], bias[0:O, :])

        # wsamp = wloc + softplus(diag) * eps  (the b == a term of L @ eps)
        nc.vector.tensor_mul(tmp[:], sp[:], epsw[:])
        nc.vector.tensor_add(wsamp[:], wloc[:], tmp[:])

        # += strict-lower part: for each b,
        #   wsamp[o,(i,a)] += wl[o,(i,a*9+b)] * eps[o,(i,b)]  for a in b+1..8.
        # Restricting each view to a > b IS the tril(-1) mask.
        for b in range(D - 1):
            na = D - 1 - b  # taps strictly below the diagonal
            wl_b = bass.AP(
                tensor=wl[:].tensor,
                offset=wl[:].offset + (b + 1) * D + b,
                ap=[list(p) for p in wl[:].ap[:1]] + [[D * D, I], [D, na]],
            )
            eps_b = bass.AP(
                tensor=epsw[:].tensor,
                offset=epsw[:].offset + b,
                ap=[list(p) for p in epsw[:].ap[:1]] + [[D, I], [0, na]],
            )
            tmp_b = bass.AP(
                tensor=tmp[:].tensor,
                offset=tmp[:].offset + b + 1,
                ap=[list(p) for p in tmp[:].ap[:1]] + [[D, I], [1, na]],
            )
            ws_b = bass.AP(
                tensor=wsamp[:].tensor,
                offset=wsamp[:].offset + b + 1,
                ap=[list(p) for p in wsamp[:].ap[:1]] + [[D, I], [1, na]],
            )
            nc.vector.tensor_tensor(tmp_b, wl_b, eps_b, mybir.AluOpType.mult)
            nc.vector.tensor_add(ws_b, ws_b, tmp_b)

        # build the 9 block-diagonal lhsT tiles:
        #   wts[:, a*128:(a+1)*128] = [[T_a, 0], [0, T_a]],  T_a[i,o] = wsamp[o, i*9+a]
        with tc.tile_pool(name="pt", bufs=1, space="PSUM") as ptp:
            # transpose the 9 taps, packed 5 + 4 into two PSUM banks, then
            # two strided copies into the lhsT tile (disjoint column ranges:
            # start=True only on the first write of each bank)
            ptA = ptp.tile([O, 5 * O], F32, name="ptA")
            ptB = ptp.tile([O, 4 * O], F32, name="ptB")
            for a in range(D):
                w_a = bass.AP(
                    tensor=wsamp[:].tensor,
                    offset=wsamp[:].offset + a,
                    ap=[list(p) for p in wsamp[:].ap[:1]] + [[D, I]],
                )
                dst_pt = ptA if a < 5 else ptB
                c = a if a < 5 else a - 5
                nc.tensor.matmul(
                    dst_pt[:, c * O : (c + 1) * O],
                    w_a,
                    ident[:],
                    is_transpose=True,
                    start=(c == 0),
                    stop=(c == (4 if a < 5 else 3)),
                    skip_group_check=True,
                )
            for pt_t, a0, na_t in ((ptA, 0, 5), (ptB, 5, 4)):
                dst = bass.AP(
                    tensor=wts[0:O].tensor,
                    offset=wts[0:O].offset + a0 * 128,
                    ap=[list(p) for p in wts[0:O].ap[:1]] + [[128, na_t], [1, O]],
                )
                nc.vector.tensor_copy(dst, pt_t[:].rearrange("p (a o) -> p a o", o=O))
        # partition-shifted copy of the diagonal blocks: [0:64, a*128:+64] ->
        # [64:128, a*128+64:+64]
        wts_lo = wts[0:O]
        wts_hi = wts[O:128]
        src = bass.AP(
            tensor=wts_lo.tensor,
            offset=wts_lo.offset,
            ap=[list(p) for p in wts_lo.ap[:1]] + [[128, D], [1, O]],
        )
        dst = bass.AP(
            tensor=wts_hi.tensor,
            offset=wts_hi.offset + O,
            ap=[list(p) for p in wts_hi.ap[:1]] + [[128, D], [1, O]],
        )
        nc.scalar.dma_start(dst, src)

        # ---------------- convolution ---------------------------------------
        xp = stack.enter_context(tc.tile_pool(name="xstrip", bufs=x_bufs))
        op = stack.enter_context(tc.tile_pool(name="ostrip", bufs=o_bufs))
        pp = stack.enter_context(tc.tile_pool(name="acc", bufs=8, space="PSUM"))
        for _pass in range(passes):
            for pair in range(nb // 2):
                n0 = 2 * pair
                strips = [(s * rstrip, rstrip) for s in range(nstrips)]
                if pair == nb // 2 - 1 and _pass == passes - 1 and rstrip >= 8:
                    # Taper the final strips so the kernel does not end on a
                    # full-size store DMA the PE has to wait out.
                    h_last = strips.pop()[0]
                    r = rstrip
                    while r > 4:
                        r1 = (r // 2 + 1) & ~1
                        strips.append((h_last, r1))
                        h_last += r1
                        r -= r1
                    strips.append((h_last, r))
                for h0, rout in strips:
                    xs = xp.tile([128, rstrip + 2, wpad], F32R, name="xs")
                    # zero the left/right halo columns
                    halo = bass.AP(
                        tensor=xs[:].tensor,
                        offset=xs[:].offset,
                        ap=[list(p) for p in xs[:].ap[:1]]
                        + [[wpad, rout + 2], [ww + 1, 2]],
                    )
                    nc.gpsimd.memset(halo.bitcast(F32), 0.0)
                    # load input rows [h0-1, h0+rout], clipped to the image
                    r_lo = max(h0 - 1, 0)
                    r_hi = min(h0 + rout + 1, hh)
                    dst_r0 = r_lo - (h0 - 1)
                    if h0 == 0:
                        nc.gpsimd.memset(xs[:, 0:1, :].bitcast(F32), 0.0)
                    if h0 + rout == hh:
                        nc.gpsimd.memset(
                            xs[:, rout + 1 : rout + 2, :].bitcast(F32), 0.0
                        )
                    src = x_t[n0 : n0 + 2, :, r_lo:r_hi, :].rearrange(
                        "n i h w -> (n i) h w"
                    )
                    nc.sync.dma_start(
                        xs[:, dst_r0 : dst_r0 + (r_hi - r_lo), 1 : ww + 1], src
                    )

                    os_ = op.tile([128, rout, ww], F32, name="os_")
                    for j in range(rout // 2):
                        acc = pp.tile([128, 2, ww], F32, name="acc")
                        for tap in range(D):
                            dy, dx = tap // 3 - 1, tap % 3 - 1
                            rhs = bass.AP(
                                tensor=xs[:].tensor,
                                offset=xs[:].offset
                                + (2 * j + 1 + dy) * wpad
                                + 1
                                + dx,
                                ap=[list(p) for p in xs[:].ap[:1]]
                                + [[wpad, 2], [1, ww]],
                            )
                            nc.tensor.matmul(
                                acc[:],
                                wts[:, tap * 128 : (tap + 1) * 128],
                                rhs,
                                start=(tap == 0),
                                stop=(tap == D - 1),
                            )
                        nc.scalar.activation(
                            os_[:, 2 * j : 2 * j + 2, :],
                            acc[:],
                            AF.Identity,
                            bias=bias[:, 0:1],
                        )
                    dst = out_t[n0 : n0 + 2, :, h0 : h0 + rout, :].rearrange(
                        "n i h w -> (n i) h w"
                    )
                    nc.sync.dma_start(dst, os_[:])

    nc.compile()
    return nc


_CACHED_NC = None


def _host_inputs(x_shard, weight_loc, weight_L, bias_loc, bias_ro, eps_w, eps_b):
    return {
        "x": np.ascontiguousarray(x_shard, np.float32),
        "wL": np.ascontiguousarray(weight_L.reshape(O, I * D * D), np.float32),
        "wloc": np.ascontiguousarray(weight_loc.reshape(O, I * D), np.float32),
        "epsw": np.ascontiguousarray(eps_w.reshape(O, I * D), np.float32),
        "ident": np.eye(O, dtype=np.float32),
        "bias3": np.ascontiguousarray(
            np.stack([bias_loc, bias_ro, eps_b]).astype(np.float32)
        ),
    }


def kernel(x, weight_loc, weight_L, bias_loc, bias_ro, eps_w, eps_b):
    global _CACHED_NC
    from concourse.bass_utils import run_bass_kernel_spmd

    x = np.asarray(x, np.float32)
    nb = x.shape[0] // N_CORES
    if _CACHED_NC is None:
        _CACHED_NC = build_nc(nb=nb)
    nc = _CACHED_NC

    in_maps = [
        _host_inputs(
            x[c * nb : (c + 1) * nb],
            np.asarray(weight_loc),
            np.asarray(weight_L),
            np.asarray(bias_loc),
            np.asarray(bias_ro),
            np.asarray(eps_w),
            np.asarray(eps_b),
        )
        for c in range(N_CORES)
    ]
    res = run_bass_kernel_spmd(nc, in_maps, list(range(N_CORES)))
    return np.concatenate([res.results[c]["out"] for c in range(N_CORES)], axis=0)
